# revision 20
# baseline (speedup 1.0000x reference)
"""CNN-MRF loss (retrieval kNN) on 8 Trainium2 NeuronCores.

Reference: cosine-similarity argmax between all 96x96 content patches and
96x96 style patches (3x3xC=128 patches, d=1152), gather matched style
patches, fold (overlap-add), MSE against content features.

Sharding: content-patch axis N split 8 ways (12 grid rows / core), style
replicated.  Per core:
  coarse: fp8(e4m3) similarity via DoubleRow matmuls.  The style side is
     pre-scaled on host by 1/||s_m|| (and a global x32 for fp8 range), so
     PSUM directly holds the scaled cosine scores.  Contraction D=1152 =
     9 channel-shifts of 128; shifts are paired into 4 DoubleRow matmuls
     (contraction 256 each, 2x PE rate) + 1 plain fp8 matmul.
  argmax: ACT copies PSUM -> SBUF bf16; DVE max8 + find_index8 give the
     best style patch per content patch.  fp8 quantization flips some
     near-tie argmaxes; the MSE is insensitive to those (verified
     rel err ~1e-3 << 2e-2).
  fold: indirect-DMA gather of matched (un-normalized bf16) style patch
     rows, then accumulating matmuls (lhsT=matched rows, rhs=identity
     columns) transpose them to channel-major directly INTO a persistent
     PSUM accumulator -- no DVE adds.  Fold for tile j-1 is issued after
     tile j's coarse matmuls so the PE never stalls on the argmax chain.
Host: sums the 8 overlapping strips, divides by fold counts, MSE.
"""
import sys
import numpy as np

for _p in ("/opt/trn_rl_repo",):
    if _p not in sys.path:
        sys.path.insert(0, _p)

import concourse.bass as bass
import concourse.bacc as bacc
import concourse.mybir as mybir
from concourse.bass import IndirectOffsetOnAxis
from concourse.bass_utils import run_bass_kernel_spmd
from concourse.tile import TileContext
from concourse.masks import make_identity

F32 = mybir.dt.float32
BF16 = mybir.dt.bfloat16
F8 = mybir.dt.float8e4
U32 = mybir.dt.uint32
DR = mybir.MatmulPerfMode.DoubleRow

C = 128          # channels
H = W = 96       # feature-map spatial dims
PW = 3           # patch size
HP = H + 2       # padded spatial
N = H * W        # content patches total (9216)
M = N            # style patches (9216)
D = C * PW * PW  # patch vector length (1152)
NCORES = 8
RPC = H // NCORES       # content grid rows per core (12)
NSH = RPC * W           # content patches per core (1152)
NT = NSH // 128         # n-tiles of 128 per core (9)
MTW = 512               # m-tile width
NMT = M // MTW          # m-tiles (18)
GRP = 2                 # m-tiles per PSUM group
SCALE = 32.0            # global fp8 style scale (argmax-invariant)
RW = 128                # racc row stride (power of 2: rows never straddle
                        # a 2KB PSUM bank)
# ss chunks: split the [C, 9, M] style tensor into column blocks so the
# first matmuls only wait on the first (small) chunk's DMA
SS_CHUNKS = [(0, 1024), (1024, 2048), (3072, 2048), (5120, 2048), (7168, 2048)]


def ts(i, size):
    return slice(i * size, (i + 1) * size)


def _chunk_of(mt):
    """m-tile index -> (chunk index, column offset within chunk)."""
    off = mt * MTW
    for ci, (o, w) in enumerate(SS_CHUNKS):
        if o <= off < o + w:
            return ci, off - o
    raise AssertionError(mt)


def build_program():
    nc = bacc.Bacc()

    cshift8 = nc.declare_dram_parameter("cshift8", [C, 9, NSH], F8, isOutput=False)
    ss8 = nc.declare_dram_parameter("ss8", [C, 9, M], F8, isOutput=False)
    sprows = nc.declare_dram_parameter("sprows", [M, D], BF16, isOutput=False)
    idx_out = nc.declare_dram_parameter("idx_out", [NT, 128, 1], U32, isOutput=True)
    racc_out = nc.declare_dram_parameter("racc_out", [C, 14 * RW], F32, isOutput=True)

    with TileContext(nc) as tc:
        with (
            tc.tile_pool(name="const", bufs=1) as constp,
            tc.tile_pool(name="big", bufs=1) as bigp,
            tc.tile_pool(name="work", bufs=2) as workp,
            tc.tile_pool(name="psS", bufs=4, space="PSUM") as psS,
            tc.tile_pool(name="psR", bufs=1, space="PSUM") as psR,
        ):
            ident = constp.tile([128, 128], BF16)
            make_identity(nc, ident[:])
            zrow = constp.tile([128, 512], BF16)
            nc.vector.memset(zrow[:], 0.0)

            # cshift in 3 tiles of 3 j's each (j=0 only needs the first),
            # interleaved with the ss chunks so the first coarse matmuls
            # start ~4.5us in
            csh_t = []
            csh_t.append(bigp.tile([C, 9, 384], F8, name="csh_0"))
            nc.sync.dma_start(out=csh_t[0][:], in_=cshift8[:, :, 0:384])
            ss_t = []
            for ci, (o, w) in enumerate(SS_CHUNKS):
                t = bigp.tile([C, 9, w], F8, name=f"ss_{ci}")
                nc.sync.dma_start(out=t[:], in_=ss8[:, :, o : o + w])
                ss_t.append(t)
                if ci == 0:
                    for pi, p in enumerate((384, 768)):
                        ct = bigp.tile([C, 9, 384], F8, name=f"csh_{pi+1}")
                        nc.sync.dma_start(out=ct[:], in_=cshift8[:, :, p : p + 384])
                        csh_t.append(ct)

            # persistent fold accumulator in PSUM: [C, 14 rows x RW]
            racc_ps = psR.tile([128, 14 * RW], F32)
            # zero it (and set has_written) with bank-aligned zero-matmuls
            for o, w in ((0, 512), (512, 512), (1024, 512), (1536, 256)):
                nc.tensor.matmul(
                    out=racc_ps[:, o : o + w],
                    lhsT=zrow[:, 0:128],
                    rhs=zrow[:, 0:w],
                    start=True,
                    stop=True,
                    skip_group_check=True,
                )

            def fold(j, matched):
                """Accumulate matched style rows (n-major) into racc_ps,
                transposed to channel-major via identity-matmuls."""
                mm3 = matched[:].rearrange("p (k c) -> p k c", c=128)
                n0 = j * 128
                r0, cc0 = n0 // W, n0 % W
                ln1 = W - cc0
                for k in range(9):
                    ki, kj = k // 3, k % 3
                    lhsT = mm3[:, k, :]
                    o1 = (r0 + ki) * RW + cc0 + kj
                    nc.tensor.matmul(
                        out=racc_ps[:, o1 : o1 + ln1],
                        lhsT=lhsT,
                        rhs=ident[:, 0:ln1],
                        start=False,
                        stop=True,
                        skip_group_check=True,
                    )
                    o2 = (r0 + 1 + ki) * RW + kj
                    nc.tensor.matmul(
                        out=racc_ps[:, o2 : o2 + 128 - ln1],
                        lhsT=lhsT,
                        rhs=ident[:, ln1:128],
                        start=False,
                        stop=True,
                        skip_group_check=True,
                    )

            NBLK = M // 128  # 72 argmax blocks of 128
            pending = []  # [(j, matched)] awaiting fold (2-deep pipeline)
            for j in range(NT):
                last = j == NT - 1
                S_sb = bigp.tile([128, M], BF16, tag="S_sb", bufs=3)
                mu = None
                if last:
                    mu = workp.tile([128, NBLK], BF16, tag="mu", name="mu")
                for g in range(0, NMT, GRP):
                    pts = []
                    for t in range(GRP):
                        pts.append(
                            psS.tile([128, MTW], F32, tag="psS", name=f"ps_{j}_{g+t}")
                        )
                    for kp in range(4):
                        lhsT = csh_t[j // 3][:, 2 * kp : 2 * kp + 2, ts(j % 3, 128)]
                        for t in range(GRP):
                            ci, lo = _chunk_of(g + t)
                            nc.tensor.matmul(
                                out=pts[t][:],
                                lhsT=lhsT,
                                rhs=ss_t[ci][:, 2 * kp : 2 * kp + 2, lo : lo + MTW],
                                start=(kp == 0),
                                stop=False,
                                perf_mode=DR,
                                skip_group_check=True,
                            )
                    lhsT8 = csh_t[j // 3][:, 8, ts(j % 3, 128)]
                    for t in range(GRP):
                        ci, lo = _chunk_of(g + t)
                        nc.tensor.matmul(
                            out=pts[t][:],
                            lhsT=lhsT8,
                            rhs=ss_t[ci][:, 8, lo : lo + MTW],
                            start=False,
                            stop=True,
                            skip_group_check=True,
                        )
                    for t in range(GRP):
                        mt = g + t
                        nc.scalar.copy(S_sb[:, ts(mt, MTW)], pts[t][:])
                        if last:
                            # block maxes pipelined behind the ACT copies,
                            # to shorten the exposed tail chain
                            nc.vector.tensor_reduce(
                                out=mu[:, mt * 4 : mt * 4 + 4],
                                in_=S_sb[:, ts(mt, MTW)].rearrange(
                                    "p (b i) -> p b i", i=128
                                ),
                                axis=mybir.AxisListType.X,
                                op=mybir.AluOpType.max,
                            )
                    if last and g == 8:  # m-tiles 0..9 done, half-1 covered
                        # first-half index pass while the second half's
                        # matmuls run
                        mh1 = workp.tile([128, 8], BF16, tag="mh1")
                        nc.vector.max(mh1[:], mu[:, : NBLK // 2])
                        ih1 = workp.tile([128, 8], U32, tag="ih1")
                        nc.vector.max_index(ih1[:], mh1[:], S_sb[:, : M // 2])

                # fold a previous tile now -- its gather completed while
                # this tile's matmuls ran, so the PE never waits
                if len(pending) >= 2:
                    fold(*pending.pop(0))

                bestu = workp.tile([128, 1], U32, tag="bestu")
                if not last:
                    # latency is hidden by the 2-deep fold pipeline: plain
                    # full-width max8 + find_index8
                    m8 = workp.tile([128, 8], BF16, tag="m8")
                    nc.vector.max(m8[:], S_sb[:])
                    idx8 = workp.tile([128, 8], U32, tag="idx8")
                    nc.vector.max_index(idx8[:], m8[:], S_sb[:])
                    nc.vector.tensor_copy(bestu[:], idx8[:, 0:1])
                else:
                    # second-half index pass + select-merge with the first
                    mh2 = workp.tile([128, 8], BF16, tag="mh2")
                    nc.vector.max(mh2[:], mu[:, NBLK // 2 :])
                    ih2 = workp.tile([128, 8], U32, tag="ih2")
                    nc.vector.max_index(ih2[:], mh2[:], S_sb[:, M // 2 :])
                    i1f = workp.tile([128, 1], F32, tag="i1f")
                    nc.vector.tensor_copy(i1f[:], ih1[:, 0:1])
                    i2f = workp.tile([128, 1], F32, tag="i2f")
                    nc.vector.tensor_scalar_add(i2f[:], ih2[:, 0:1], float(M // 2))
                    sel = workp.tile([128, 1], mybir.dt.uint8, tag="sel")
                    nc.vector.tensor_tensor(
                        out=sel[:],
                        in0=mh1[:, 0:1],
                        in1=mh2[:, 0:1],
                        op=mybir.AluOpType.is_ge,
                    )
                    bestf = workp.tile([128, 1], F32, tag="bestf")
                    nc.vector.select(bestf[:], sel[:], i1f[:], i2f[:])
                    nc.vector.tensor_copy(bestu[:], bestf[:])

                matched = workp.tile([128, D], BF16, tag="matched", bufs=3)
                nc.gpsimd.indirect_dma_start(
                    out=matched[:],
                    out_offset=None,
                    in_=sprows[:],
                    in_offset=IndirectOffsetOnAxis(ap=bestu[:, 0:1], axis=0),
                )
                nc.sync.dma_start(out=idx_out[j], in_=bestu[:])
                pending.append((j, matched))

            for p in pending:
                fold(*p)
            racc_sb = bigp.tile([128, 14 * RW], F32, name="racc_sb")
            for i, o in enumerate(range(0, 14 * RW, 512)):
                w = min(512, 14 * RW - o)
                eng = nc.scalar if i % 2 == 0 else nc.vector
                if eng is nc.scalar:
                    eng.copy(racc_sb[:, o : o + w], racc_ps[:, o : o + w])
                else:
                    eng.tensor_copy(racc_sb[:, o : o + w], racc_ps[:, o : o + w])
            nc.sync.dma_start(out=racc_out[:], in_=racc_sb[:])

    if not nc.is_finalized():
        nc.finalize()
    return nc


_PROGRAM = None


def _get_program():
    global _PROGRAM
    if _PROGRAM is None:
        _PROGRAM = build_program()
    return _PROGRAM


def _host_prep(content_feats, style_feats):
    """Build per-core input maps."""
    f8 = mybir.dt.np(F8)
    bf = mybir.dt.np(BF16)
    cf = np.ascontiguousarray(np.asarray(content_feats, dtype=np.float32)[0])
    sf = np.ascontiguousarray(np.asarray(style_feats, dtype=np.float32)[0])
    cpad = np.pad(cf, ((0, 0), (1, 1), (1, 1)))
    spad = np.pad(sf, ((0, 0), (1, 1), (1, 1)))

    # style patch rows in (ki, kj, c) order, un-normalized, bf16 (for the
    # matched-row gather + fold)
    w = np.lib.stride_tricks.sliding_window_view(spad, (PW, PW), axis=(1, 2))
    # w: (C, 96, 96, 3, 3) -> (96, 96, 3, 3, C) -> (M, 9*C)
    sprows_kc = np.ascontiguousarray(
        w.transpose(1, 2, 3, 4, 0).reshape(M, PW * PW * C).astype(bf)
    )
    # norms from the (c,ki,kj) rows (same values, order irrelevant)
    nrm = np.linalg.norm(
        w.transpose(1, 2, 3, 4, 0).reshape(M, -1).astype(np.float64), axis=1
    )
    invn = (SCALE / np.maximum(nrm, 1e-12)).astype(np.float32)

    # pre-scaled shifted style maps: ss8[c, k, m] = spad[c,mi+ki,mj+kj]*invn[m]
    ss = np.empty((C, 9, M), dtype=np.float32)
    for k in range(9):
        ki, kj = k // 3, k % 3
        ss[:, k, :] = spad[:, ki : ki + H, kj : kj + W].reshape(C, M)
    ss *= invn[None, None, :]
    ss8 = np.ascontiguousarray(ss.astype(f8))

    in_maps = []
    for i in range(NCORES):
        slab = cpad[:, i * RPC : i * RPC + RPC + 2, :]  # (C, 14, 98)
        csh = np.empty((C, 9, NSH), dtype=np.float32)
        for k in range(9):
            ki, kj = k // 3, k % 3
            csh[:, k, :] = slab[:, ki : ki + RPC, kj : kj + W].reshape(C, NSH)
        in_maps.append(
            {
                "cshift8": np.ascontiguousarray(csh.astype(f8)),
                "ss8": ss8,
                "sprows": sprows_kc,
            }
        )
    return cf, in_maps


_DIVISOR = None


def _fold_divisor():
    global _DIVISOR
    if _DIVISOR is None:
        cnt = np.full(H, 3, dtype=np.float32)
        cnt[0] = cnt[-1] = 2
        _DIVISOR = np.outer(cnt, cnt).astype(np.float32) + np.float32(1e-8)
    return _DIVISOR


def _host_combine(cf, results):
    acc = np.zeros((C, H + 2, W), dtype=np.float32)
    for i in range(NCORES):
        strip = results[i]["racc_out"].reshape(C, 14, RW)[:, :, 1 : 1 + W]
        acc[:, i * RPC : i * RPC + RPC + 2, :] += strip
    recon = acc[:, 1 : 1 + H, :] / _fold_divisor()[None, :, :]
    diff = cf - recon
    return np.float32(np.mean(np.square(diff), dtype=np.float64))


def run(content_feats, style_feats, trace=False):
    nc = _get_program()
    cf, in_maps = _host_prep(content_feats, style_feats)
    res = run_bass_kernel_spmd(
        nc, in_maps, core_ids=list(range(NCORES)), trace=trace
    )
    mse = _host_combine(cf, res.results)
    return mse, res


def kernel(content_feats, style_feats):
    mse, _ = run(content_feats, style_feats)
    return np.array(mse, dtype=np.float32)


# revision 29
# speedup vs baseline: 1.0746x; 1.0746x over previous
"""CNN-MRF loss (retrieval kNN) on 8 Trainium2 NeuronCores.

Reference: cosine-similarity argmax between all 96x96 content patches and
96x96 style patches (3x3xC=128 patches, d=1152), gather matched style
patches, fold (overlap-add), MSE against content features.

Sharding: content-patch axis N split 8 ways (12 grid rows / core), style
replicated.  Per core:
  coarse: fp8(e4m3) similarity via DoubleRow matmuls.  The style side is
     pre-scaled on host by 1/||s_m|| (and a global x32 for fp8 range), so
     PSUM directly holds the scaled cosine scores.  Contraction D=1152 =
     9 channel-shifts of 128; shifts are paired into 4 DoubleRow matmuls
     (contraction 256 each, 2x PE rate) + 1 plain fp8 matmul.
  argmax: ACT copies PSUM -> SBUF bf16; DVE max8 + find_index8 give the
     best style patch per content patch.  fp8 quantization flips some
     near-tie argmaxes; the MSE is insensitive to those (verified
     rel err ~1e-3 << 2e-2).
  fold: indirect-DMA gather of matched (un-normalized bf16) style patch
     rows, then accumulating matmuls (lhsT=matched rows, rhs=identity
     columns) transpose them to channel-major directly INTO a persistent
     PSUM accumulator -- no DVE adds.  Fold for tile j-1 is issued after
     tile j's coarse matmuls so the PE never stalls on the argmax chain.
Host: sums the 8 overlapping strips, divides by fold counts, MSE.
"""
import sys
import numpy as np

for _p in ("/opt/trn_rl_repo",):
    if _p not in sys.path:
        sys.path.insert(0, _p)

import concourse.bass as bass
import concourse.bacc as bacc
import concourse.mybir as mybir
from concourse.bass import IndirectOffsetOnAxis
from concourse.bass_utils import run_bass_kernel_spmd
from concourse.tile import TileContext
from concourse.masks import make_identity

F32 = mybir.dt.float32
BF16 = mybir.dt.bfloat16
F8 = mybir.dt.float8e4
U32 = mybir.dt.uint32
DR = mybir.MatmulPerfMode.DoubleRow

C = 128          # channels
H = W = 96       # feature-map spatial dims
PW = 3           # patch size
HP = H + 2       # padded spatial
N = H * W        # content patches total (9216)
M = N            # style patches (9216)
D = C * PW * PW  # patch vector length (1152)
NCORES = 8
RPC = H // NCORES       # content grid rows per core (12)
NSH = RPC * W           # content patches per core (1152)
NT = NSH // 128         # n-tiles of 128 per core (9)
MTW = 512               # m-tile width
NMT = M // MTW          # m-tiles (18)
GRP = 2                 # m-tiles per PSUM group
SCALE = 32.0            # global fp8 style scale (argmax-invariant)
RW = 128                # racc row stride (power of 2: rows never straddle
                        # a 2KB PSUM bank)
# ss chunks: split the [C, 9, M] style tensor into column blocks so the
# first matmuls only wait on the first (small) chunk's DMA
SS_CHUNKS = [(0, 1024), (1024, 2048), (3072, 2048), (5120, 2048), (7168, 2048)]


def ts(i, size):
    return slice(i * size, (i + 1) * size)


def _chunk_of(mt):
    """m-tile index -> (chunk index, column offset within chunk)."""
    off = mt * MTW
    for ci, (o, w) in enumerate(SS_CHUNKS):
        if o <= off < o + w:
            return ci, off - o
    raise AssertionError(mt)


def build_program():
    nc = bacc.Bacc()

    cshift8 = nc.declare_dram_parameter("cshift8", [C, 9, NSH], F8, isOutput=False)
    ss8 = nc.declare_dram_parameter("ss8", [C, 9, M], F8, isOutput=False)
    sprows = nc.declare_dram_parameter("sprows", [M, D], BF16, isOutput=False)
    sprows8 = nc.declare_dram_parameter("sprows8", [M, D], F8, isOutput=False)
    idx_out = nc.declare_dram_parameter("idx_out", [NT, 128, 1], U32, isOutput=True)
    racc_out = nc.declare_dram_parameter("racc_out", [C, 14 * RW], F32, isOutput=True)

    with TileContext(nc) as tc:
        with (
            tc.tile_pool(name="const", bufs=1) as constp,
            tc.tile_pool(name="big", bufs=1) as bigp,
            tc.tile_pool(name="work", bufs=2) as workp,
            tc.tile_pool(name="psS", bufs=4, space="PSUM") as psS,
            tc.tile_pool(name="psR", bufs=1, space="PSUM") as psR,
        ):
            ident = constp.tile([128, 128], BF16)
            make_identity(nc, ident[:])
            ident8 = constp.tile([128, 128], F8)
            make_identity(nc, ident8[:])
            zrow = constp.tile([128, 512], BF16)
            nc.vector.memset(zrow[:], 0.0)

            # cshift in 3 tiles of 3 j's each (j=0 only needs the first),
            # interleaved with the ss chunks so the first coarse matmuls
            # start ~4.5us in
            csh_t = []
            csh_t.append(bigp.tile([C, 9, 384], F8, name="csh_0"))
            nc.sync.dma_start(out=csh_t[0][:], in_=cshift8[:, :, 0:384])
            ss_t = []
            for ci, (o, w) in enumerate(SS_CHUNKS):
                t = bigp.tile([C, 9, w], F8, name=f"ss_{ci}")
                nc.sync.dma_start(out=t[:], in_=ss8[:, :, o : o + w])
                ss_t.append(t)
                if ci == 0:
                    for pi, p in enumerate((384, 768)):
                        ct = bigp.tile([C, 9, 384], F8, name=f"csh_{pi+1}")
                        nc.sync.dma_start(out=ct[:], in_=cshift8[:, :, p : p + 384])
                        csh_t.append(ct)

            # persistent fold accumulator in PSUM: [C, 14 rows x RW]
            racc_ps = psR.tile([128, 14 * RW], F32)
            # zero it (and set has_written) with bank-aligned zero-matmuls
            for o, w in ((0, 512), (512, 512), (1024, 512), (1536, 256)):
                nc.tensor.matmul(
                    out=racc_ps[:, o : o + w],
                    lhsT=zrow[:, 0:128],
                    rhs=zrow[:, 0:w],
                    start=True,
                    stop=True,
                    skip_group_check=True,
                )

            def fold(j, matched, idn):
                """Accumulate matched style rows (n-major) into racc_ps,
                transposed to channel-major via identity-matmuls."""
                mm3 = matched[:].rearrange("p (k c) -> p k c", c=128)
                n0 = j * 128
                r0, cc0 = n0 // W, n0 % W
                ln1 = W - cc0
                for k in range(9):
                    ki, kj = k // 3, k % 3
                    lhsT = mm3[:, k, :]
                    o1 = (r0 + ki) * RW + cc0 + kj
                    nc.tensor.matmul(
                        out=racc_ps[:, o1 : o1 + ln1],
                        lhsT=lhsT,
                        rhs=idn[:, 0:ln1],
                        start=False,
                        stop=True,
                        skip_group_check=True,
                    )
                    o2 = (r0 + 1 + ki) * RW + kj
                    nc.tensor.matmul(
                        out=racc_ps[:, o2 : o2 + 128 - ln1],
                        lhsT=lhsT,
                        rhs=idn[:, ln1:128],
                        start=False,
                        stop=True,
                        skip_group_check=True,
                    )

            def tree72(src, width, tagp):
                """Pairwise tensor_tensor max tree (2x bf16 DVE rate) from
                `width` down to 72 block maxes; every output IS an S value."""
                sA = workp.tile([128, width // 2], BF16, tag=f"{tagp}A",
                                name=f"{tagp}A", bufs=1)
                sB = workp.tile([128, width // 4], BF16, tag=f"{tagp}B",
                                name=f"{tagp}B", bufs=1)
                w = width // 2
                nc.vector.tensor_tensor(
                    out=sA[:, :w], in0=src[:, :w], in1=src[:, w:],
                    op=mybir.AluOpType.max,
                )
                cur, other = sA, sB
                while w > 72:
                    nw = w // 2
                    nc.vector.tensor_tensor(
                        out=other[:, :nw], in0=cur[:, :nw], in1=cur[:, nw:w],
                        op=mybir.AluOpType.max,
                    )
                    cur, other = other, cur
                    w = nw
                return cur[:, :w]

            pending = []  # [(j, matched, ident)] awaiting fold (2-deep)
            for j in range(NT):
                last = j == NT - 1
                S_sb = bigp.tile([128, M], BF16, tag="S_sb", bufs=2)
                for g in range(0, NMT, GRP):
                    pts = []
                    for t in range(GRP):
                        pts.append(
                            psS.tile([128, MTW], F32, tag="psS", name=f"ps_{j}_{g+t}")
                        )
                    for kp in range(4):
                        lhsT = csh_t[j // 3][:, 2 * kp : 2 * kp + 2, ts(j % 3, 128)]
                        for t in range(GRP):
                            ci, lo = _chunk_of(g + t)
                            nc.tensor.matmul(
                                out=pts[t][:],
                                lhsT=lhsT,
                                rhs=ss_t[ci][:, 2 * kp : 2 * kp + 2, lo : lo + MTW],
                                start=(kp == 0),
                                stop=False,
                                perf_mode=DR,
                                skip_group_check=True,
                            )
                    lhsT8 = csh_t[j // 3][:, 8, ts(j % 3, 128)]
                    for t in range(GRP):
                        ci, lo = _chunk_of(g + t)
                        nc.tensor.matmul(
                            out=pts[t][:],
                            lhsT=lhsT8,
                            rhs=ss_t[ci][:, 8, lo : lo + MTW],
                            start=False,
                            stop=True,
                            skip_group_check=True,
                        )
                    for t in range(GRP):
                        mt = g + t
                        nc.scalar.copy(S_sb[:, ts(mt, MTW)], pts[t][:])
                    if last and g == 8:  # m-tiles 0..9 done, half-1 covered
                        # first-half max tree + index pass while the second
                        # half's matmuls run
                        t72h1 = tree72(S_sb[:, : M // 2], M // 2, "h1")
                        mh1 = workp.tile([128, 8], BF16, tag="mh1")
                        nc.vector.max(mh1[:], t72h1)
                        ih1 = workp.tile([128, 8], U32, tag="ih1")
                        nc.vector.max_index(ih1[:], mh1[:], S_sb[:, : M // 2])

                # fold a previous tile now -- its gather completed while
                # this tile's matmuls ran, so the PE never waits
                if len(pending) >= 2:
                    fold(*pending.pop(0))

                bestu = workp.tile([128, 1], U32, tag="bestu")
                if not last:
                    # latency is hidden by the 2-deep fold pipeline:
                    # tree-max (2x bf16 rate) + one full index pass
                    t72 = tree72(S_sb[:], M, "tr")
                    m8 = workp.tile([128, 8], BF16, tag="m8")
                    nc.vector.max(m8[:], t72)
                    idx8 = workp.tile([128, 8], U32, tag="idx8")
                    nc.vector.max_index(idx8[:], m8[:], S_sb[:])
                    nc.vector.tensor_copy(bestu[:], idx8[:, 0:1])
                else:
                    # second-half tree + index pass, then select-merge
                    t72h2 = tree72(S_sb[:, M // 2 :], M // 2, "h2")
                    mh2 = workp.tile([128, 8], BF16, tag="mh2")
                    nc.vector.max(mh2[:], t72h2)
                    ih2 = workp.tile([128, 8], U32, tag="ih2")
                    nc.vector.max_index(ih2[:], mh2[:], S_sb[:, M // 2 :])
                    i1f = workp.tile([128, 1], F32, tag="i1f")
                    nc.vector.tensor_copy(i1f[:], ih1[:, 0:1])
                    i2f = workp.tile([128, 1], F32, tag="i2f")
                    nc.vector.tensor_scalar_add(i2f[:], ih2[:, 0:1], float(M // 2))
                    sel = workp.tile([128, 1], mybir.dt.uint8, tag="sel")
                    nc.vector.tensor_tensor(
                        out=sel[:],
                        in0=mh1[:, 0:1],
                        in1=mh2[:, 0:1],
                        op=mybir.AluOpType.is_ge,
                    )
                    bestf = workp.tile([128, 1], F32, tag="bestf")
                    nc.vector.select(bestf[:], sel[:], i1f[:], i2f[:])
                    nc.vector.tensor_copy(bestu[:], bestf[:])

                if not last:
                    matched = workp.tile([128, D], BF16, tag="matched", bufs=3)
                    nc.gpsimd.indirect_dma_start(
                        out=matched[:],
                        out_offset=None,
                        in_=sprows[:],
                        in_offset=IndirectOffsetOnAxis(ap=bestu[:, 0:1], axis=0),
                    )
                    pending.append((j, matched, ident))
                else:
                    # exposed tail gather: fp8 rows (half the DMA time);
                    # fp8 fold error on 1/72 of patches is ~5e-5 rel MSE
                    matched8 = workp.tile([128, D], F8, tag="matched8")
                    nc.gpsimd.indirect_dma_start(
                        out=matched8[:],
                        out_offset=None,
                        in_=sprows8[:],
                        in_offset=IndirectOffsetOnAxis(ap=bestu[:, 0:1], axis=0),
                    )
                    pending.append((j, matched8, ident8))
                nc.sync.dma_start(out=idx_out[j], in_=bestu[:])

            for p in pending:
                fold(*p)
            # stage PSUM -> SBUF in 4 independent tiles (2 on ACT, 2 on DVE,
            # no tile-level write serialization), then 4 parallel DMAs
            for i, o in enumerate(range(0, 14 * RW, 512)):
                w = min(512, 14 * RW - o)
                rt = bigp.tile([128, w], F32, name=f"racc_sb{i}")
                if i % 2 == 0:
                    nc.scalar.copy(rt[:], racc_ps[:, o : o + w])
                else:
                    nc.vector.tensor_copy(rt[:], racc_ps[:, o : o + w])
                nc.sync.dma_start(out=racc_out[:, o : o + w], in_=rt[:])

    if not nc.is_finalized():
        nc.finalize()
    return nc


_PROGRAM = None


def _get_program():
    global _PROGRAM
    if _PROGRAM is None:
        _PROGRAM = build_program()
    return _PROGRAM


def _host_prep(content_feats, style_feats):
    """Build per-core input maps."""
    f8 = mybir.dt.np(F8)
    bf = mybir.dt.np(BF16)
    cf = np.ascontiguousarray(np.asarray(content_feats, dtype=np.float32)[0])
    sf = np.ascontiguousarray(np.asarray(style_feats, dtype=np.float32)[0])
    cpad = np.pad(cf, ((0, 0), (1, 1), (1, 1)))
    spad = np.pad(sf, ((0, 0), (1, 1), (1, 1)))

    # style patch rows in (ki, kj, c) order, un-normalized, bf16 (for the
    # matched-row gather + fold)
    w = np.lib.stride_tricks.sliding_window_view(spad, (PW, PW), axis=(1, 2))
    # w: (C, 96, 96, 3, 3) -> (96, 96, 3, 3, C) -> (M, 9*C)
    sprows_f = w.transpose(1, 2, 3, 4, 0).reshape(M, PW * PW * C)
    sprows_kc = np.ascontiguousarray(sprows_f.astype(bf))
    sprows_kc8 = np.ascontiguousarray(sprows_f.astype(f8))
    # norms from the (c,ki,kj) rows (same values, order irrelevant)
    nrm = np.linalg.norm(
        w.transpose(1, 2, 3, 4, 0).reshape(M, -1).astype(np.float64), axis=1
    )
    invn = (SCALE / np.maximum(nrm, 1e-12)).astype(np.float32)

    # pre-scaled shifted style maps: ss8[c, k, m] = spad[c,mi+ki,mj+kj]*invn[m]
    ss = np.empty((C, 9, M), dtype=np.float32)
    for k in range(9):
        ki, kj = k // 3, k % 3
        ss[:, k, :] = spad[:, ki : ki + H, kj : kj + W].reshape(C, M)
    ss *= invn[None, None, :]
    ss8 = np.ascontiguousarray(ss.astype(f8))

    in_maps = []
    for i in range(NCORES):
        slab = cpad[:, i * RPC : i * RPC + RPC + 2, :]  # (C, 14, 98)
        csh = np.empty((C, 9, NSH), dtype=np.float32)
        for k in range(9):
            ki, kj = k // 3, k % 3
            csh[:, k, :] = slab[:, ki : ki + RPC, kj : kj + W].reshape(C, NSH)
        in_maps.append(
            {
                "cshift8": np.ascontiguousarray(csh.astype(f8)),
                "ss8": ss8,
                "sprows": sprows_kc,
                "sprows8": sprows_kc8,
            }
        )
    return cf, in_maps


_DIVISOR = None


def _fold_divisor():
    global _DIVISOR
    if _DIVISOR is None:
        cnt = np.full(H, 3, dtype=np.float32)
        cnt[0] = cnt[-1] = 2
        _DIVISOR = np.outer(cnt, cnt).astype(np.float32) + np.float32(1e-8)
    return _DIVISOR


def _host_combine(cf, results):
    acc = np.zeros((C, H + 2, W), dtype=np.float32)
    for i in range(NCORES):
        strip = results[i]["racc_out"].reshape(C, 14, RW)[:, :, 1 : 1 + W]
        acc[:, i * RPC : i * RPC + RPC + 2, :] += strip
    recon = acc[:, 1 : 1 + H, :] / _fold_divisor()[None, :, :]
    diff = cf - recon
    return np.float32(np.mean(np.square(diff), dtype=np.float64))


def run(content_feats, style_feats, trace=False):
    nc = _get_program()
    cf, in_maps = _host_prep(content_feats, style_feats)
    res = run_bass_kernel_spmd(
        nc, in_maps, core_ids=list(range(NCORES)), trace=trace
    )
    mse = _host_combine(cf, res.results)
    return mse, res


def kernel(content_feats, style_feats):
    mse, _ = run(content_feats, style_feats)
    return np.array(mse, dtype=np.float32)
